# revision 25
# baseline (speedup 1.0000x reference)
"""ErnieLayout self-attention on 8 Trainium2 NeuronCores (Bass/Tile).

Problem shapes (hardcoded): B=4, S=1024, H=768, NH=12, HD=64.
Sharding: core c -> (batch b = c//2, head-half hh = c%2, i.e. 6 heads).
Each core computes attention for its 6 heads of one batch element and
writes the [S, 384] column slice of that batch's output.

v3: the whole sequence axis is PERMUTED on-chip: position s = 8p + r
lives on partition p, slot r. The permutation is applied consistently
to q (X rows -> Q^T columns -> out rows) and k (K^T columns, V rows,
mask) so no compute changes -- but it makes every big HBM transfer
contiguous per partition:
  X:    SWDGE cast DMAs in 4 pieces (contiguous multi-row lines)
  rel:  per head, rel_pos and rel_2d_pos stream as TWO half-tiles
        [128, 4, 1024] fp16 each, 16 KB/partition lines, fp32->fp16 cast
        inside the SWDGE DMA engines (no fp32 staging); the rel1+rel2
        merge adds run on the otherwise-idle GPSIMD engine, scheduled a
        head ahead. The rel k-axis is pre-permuted on the host
        (make_in_maps) so contiguous strip slices match the permuted
        K^T/V tiles.
  out:  scatter rows 8p+qt (1.5 KB rows, same descriptors as natural).

Per-core algorithm (mixed precision, scores kept transposed):
  setup:  W cast fp32->fp16 in the DMA; X^T / W^T built on the PE (fp16
          transposes via identity); Q^T = (Wq_s @ X^T + bq)/8,
          K^T = Wk_s @ X^T + bk (fp16 matmuls, fp32 PSUM accumulate);
          V = X @ Wv_s^T (+ bv via DVE broadcast add), stored fp16 with
          a ones column (col 64 -> softmax denominator for free)
  per (head, qch, ktile):   [qch-outer: the first half of a head needs
          only the first rel half-tile, which halves the strip-wait]
          psum[k=128, q=512] = K^T.T @ Q^T
          psum += rel12[q,ktile]^T via matmul(lhsT=rel12_f16, rhs=I_f16)
          pT = exp(psum + maskbias[k]) -> fp16   (ACT per-partition bias;
          masked keys get FLT_MIN so exp underflows to exactly 0,
          matching the reference's FLT_MIN replacement)
  per head (PV flipped so V is the stationary operand):
          ctx^T[d|1, q-chunk] += V_aug[kt].T @ pT[kt]  over kt
          ctx^T -> SBUF fp32 eagerly (2 vpsum banks recycle fast),
          back-transposed on the PE, out = ctx * (1/denominator)
          (the PE part of head h's finalize is emitted inside head h+1's
          loop so the in-order PE stream never stalls on it)

PSUM: 6 banks rotate for score tiles, 2 for ctx^T accumulation.
Precision: fp16 carries 10 mantissa bits -> final rel err ~1e-3.
"""

import os
import sys

import numpy as np

for _p in ("/opt/trn_rl_repo",):
    if _p not in sys.path and os.path.isdir(_p):
        sys.path.append(_p)

import concourse.bass as bass
import concourse.mybir as mybir
import concourse.tile as tile
from concourse import bacc
from concourse.bass_utils import run_bass_kernel_spmd
from concourse.masks import make_identity

F32 = mybir.dt.float32
F16 = mybir.dt.float16
I32 = mybir.dt.int32
AF = mybir.ActivationFunctionType
ADD = mybir.AluOpType.add
NEG = float(np.finfo(np.float32).min)

P = 128
S = 1024
NH = 6        # heads per core
HD = 64
HIN = 768     # model dim (contraction for projections)
HOUT = NH * HD  # 384, per-core projection width
KT = S // P   # 8 key tiles
QT = S // P   # 8 query tiles
R = S // P    # 8 permutation slots (s = 8p + r)
VW = HD + 1   # 65: V columns + ones column


def _build_kernel_body(tc, aps):
    import contextlib

    nc = tc.nc
    x_ap = aps["x"]
    mask_ap = aps["mask"]
    rel1_ap = aps["rel1"]
    rel2_ap = aps["rel2"]
    out_ap = aps["out"]

    with contextlib.ExitStack() as ctx:
        const = ctx.enter_context(tc.tile_pool(name="const", bufs=1))

        ident = const.tile([P, P], F16)
        make_identity(nc, ident)
        ident32 = const.tile([P, P], F32)
        nc.vector.tensor_copy(ident32[:], ident[:])

        # long-lived tensors
        qt_pool = ctx.enter_context(tc.tile_pool(name="qT", bufs=3))
        kt_pool = ctx.enter_context(tc.tile_pool(name="kT", bufs=3))
        v_pool = ctx.enter_context(tc.tile_pool(name="v", bufs=8))

        qT = [qt_pool.tile([P, S], F16, tag="qT", name=f"qT{i}") for i in range(3)]
        kT = [kt_pool.tile([P, S], F16, tag="kT", name=f"kT{i}") for i in range(3)]
        v_tiles = [
            v_pool.tile([P, NH, VW], F16, tag="v", name=f"v{i}") for i in range(8)
        ]

        # rel half-tile pools: [128, 4, 1024] fp16, written directly by
        # SWDGE cast DMAs (fp32->fp16 in the DMA engines, 16 KB HBM lines).
        # rel2 lands in a staging half and a fp16+fp16 DVE add merges it
        # into the rel1 half in place. 8 bufs = 4 heads of runway.
        rbf_pool = ctx.enter_context(tc.tile_pool(name="rbf", bufs=8))
        r2_pool = ctx.enter_context(tc.tile_pool(name="r2h", bufs=6))

        def emit_rel_dmas(h):
            """Two half-tiles per head; emission order r1a, r1b, r2a, r2b
            so the SWDGE ring streams without semaphore stalls. The DVE
            merge adds are emitted separately (emit_rel_adds) so they
            never clog the DVE FIFO ahead of near-term work."""
            halves = []
            r2h = []
            for piece in range(2):
                halves.append(rbf_pool.tile([P, R // 2, S], F16, tag="rbf",
                                            name=f"rel{h}_{piece}"))
                r2h.append(r2_pool.tile([P, R // 2, S], F16, tag="r2h",
                                        name=f"r2h{h}_{piece}"))
            for rel_ap, dst in ((rel1_ap, halves), (rel2_ap, r2h)):
                src = rel_ap[h].rearrange("(p r) k -> p r k", p=P)
                for piece in range(2):
                    sl = slice(piece * (R // 2), (piece + 1) * (R // 2))
                    nc.gpsimd.dma_start(dst[piece][:], src[:, sl, :])
            return halves, r2h

        def emit_rel_adds(pair):
            # on GPSIMD (Pool): the otherwise-idle engine; a merge add
            # waiting on its DMA can only delay future rel dispatches,
            # never the DVE recips / V adds / ACT exps (scheduling a merge
            # add on the DVE lets the tile scheduler place it ahead of
            # ready work there, which measurably re-serializes the kernel)
            halves, r2h = pair
            for piece in range(2):
                nc.gpsimd.tensor_add(halves[piece][:], halves[piece][:],
                                     r2h[piece][:])
            return halves

        # ---------------- phase 1: load (cast in DMA), transpose, project --
        with contextlib.ExitStack() as ph1:
            x16_pool = ph1.enter_context(tc.tile_pool(name="x16", bufs=4))
            w16_pool = ph1.enter_context(tc.tile_pool(name="w16", bufs=3))
            xt_pool = ph1.enter_context(tc.tile_pool(name="xT", bufs=6))
            wt_pool = ph1.enter_context(tc.tile_pool(name="wT", bufs=18))
            psum1 = ph1.enter_context(tc.tile_pool(name="psum1", bufs=4, space="PSUM"))
            psum1b = ph1.enter_context(
                tc.tile_pool(name="psum1b", bufs=2, space="PSUM")
            )

            # X: SWDGE cast DMAs in 4 piece-tiles so the first transposes
            # start as soon as piece 0 lands; partition p holds rows
            # 8p..8p+7 (contiguous HBM lines). Piece t//2, slot t%2 is
            # q-block t.
            x_perm = x_ap.rearrange("(p r) hh -> p r hh", p=P)
            x16p = []
            for piece in range(4):
                xt_ = x16_pool.tile([P, 2, HIN], F16, tag="x16",
                                    name=f"x16_{piece}")
                nc.gpsimd.dma_start(
                    xt_[:], x_perm[:, piece * 2:piece * 2 + 2, :]
                )
                x16p.append(xt_)

            def x16(t, csl):
                return x16p[t // 2][:, t % 2, csl]

            # W tiles [128, 3, 768] fp16 (natural rows d*128+p), one cast
            # DMA per projection matrix
            w16 = {}
            for wname in ("q", "k", "v"):
                w16_t = w16_pool.tile([P, 3, HIN], F16, tag="w16",
                                      name=f"w16{wname}")
                nc.gpsimd.dma_start(
                    w16_t[:],
                    aps[f"w{wname}"].rearrange("(d p) hh -> p d hh", p=P),
                )
                w16[wname] = w16_t

            # rel DMAs for the first heads follow x/w on the SWDGE ring, so
            # phase-1 inputs get the DMA engines first; the DVE merge adds
            # for heads 0/1 are emitted at the end of phase 1 so they don't
            # clog the DVE FIFO ahead of phase-1 work
            rel_dmas = {0: emit_rel_dmas(0), 1: emit_rel_dmas(1)}

            # mask bias (permuted gather: maskb[p, kt] = mask[8p + kt])
            mask_i = const.tile([P, KT], I32)
            nc.sync.dma_start(mask_i[:], mask_ap.rearrange("(p r) -> p r", p=P))
            maskb = const.tile([P, KT], F32)
            nc.vector.tensor_copy(maskb[:], mask_i[:])
            nc.vector.tensor_scalar_mul(maskb[:], maskb[:], NEG)
            bias_sb = {}
            for wname in ("q", "k"):
                bt = const.tile([P, 3], F32, tag=f"b{wname}")
                nc.sync.dma_start(
                    bt[:], aps[f"b{wname}"].rearrange("(a p) -> p a", p=P)
                )
                if wname == "q":
                    nc.vector.tensor_scalar_mul(bt[:], bt[:], 0.125)
                bias_sb[wname] = bt
            bv_bc = const.tile([P, NH, HD], F32)
            nc.sync.dma_start(
                bv_bc[:],
                aps["bv"].rearrange("(h d) -> h d", d=HD)[None].to_broadcast(
                    (P, NH, HD)
                ),
            )

            # X^T: 6 fp16 tiles [128, 1024]; column t*128+p <-> row 8p+t.
            # t-outer within hc groups: the first transposes need only X
            # piece 0, so the PE starts ~12us earlier.
            xT = []
            for hgrp in ((0, 1, 2, 3), (4, 5)):
                pts = {hc: psum1.tile([P, S], F16, tag="xtp",
                                      name=f"xtp{hc}")
                       for hc in hgrp}
                for t in range(8):
                    for hc in hgrp:
                        nc.tensor.transpose(
                            pts[hc][:, t * P:(t + 1) * P],
                            x16(t, slice(hc * P, (hc + 1) * P)),
                            ident[:],
                        )
                for hc in hgrp:
                    xt_t = xt_pool.tile([P, S], F16, tag="xT",
                                        name=f"xT{hc}")
                    nc.scalar.copy(xt_t[:], pts[hc][:])
                    xT.append(xt_t)

            # W^T slices (fp16): wT[(w, hc)] = [128, 384]
            wT = {}
            for wname in ("q", "k", "v"):
                for hc in range(6):
                    pw = psum1b.tile([P, 512], F16, tag="ps1b", name="pw")[:, :HOUT]
                    for d in range(3):
                        nc.tensor.transpose(
                            pw[:, d * P:(d + 1) * P],
                            w16[wname][:, d, hc * P:(hc + 1) * P],
                            ident[:],
                        )
                    wt_t = wt_pool.tile([P, HOUT], F16, tag="wT")
                    nc.scalar.copy(wt_t[:], pw[:])
                    wT[(wname, hc)] = wt_t

            # Q^T, K^T projections: fp16 matmuls, fp32 PSUM
            for wname, dest, scale in (("q", qT, 0.125), ("k", kT, 1.0)):
                for d in range(3):
                    for tch in range(2):
                        pp = psum1b.tile([P, 512], F32, tag="projp")
                        for hc in range(6):
                            nc.tensor.matmul(
                                pp[:],
                                wT[(wname, hc)][:, d * P:(d + 1) * P],
                                xT[hc][:, tch * 512:(tch + 1) * 512],
                                start=(hc == 0),
                                stop=(hc == 5),
                            )
                        nc.scalar.activation(
                            dest[d][:, tch * 512:(tch + 1) * 512],
                            pp[:],
                            AF.Identity,
                            bias=bias_sb[wname][:, d:d + 1],
                            scale=scale,
                        )

            # V projection: out [k-block 128, 384] fp16 + ones column
            for t in range(8):
                pv = psum1b.tile([P, 512], F32, tag="projp", name="pv")[:, :HOUT]
                for hc in range(6):
                    nc.tensor.matmul(
                        pv[:],
                        xT[hc][:, t * P:(t + 1) * P],
                        wT[("v", hc)][:],
                        start=(hc == 0),
                        stop=(hc == 5),
                    )
                nc.vector.memset(v_tiles[t][:], 1.0)
                # copy + bias add (bv broadcast along partitions)
                nc.vector.tensor_add(
                    v_tiles[t][:, :, 0:HD],
                    pv[:].rearrange("p (h d) -> p h d", d=HD),
                    bv_bc[:],
                )

            # merge adds for heads 0/1 (their DMAs landed during phase 1)
            rel_strips = {hh_: emit_rel_adds(rel_dmas.pop(hh_))
                          for hh_ in (0, 1)}

        # ---------------- phase 2: attention per head ----------------
        out_pool = ctx.enter_context(tc.tile_pool(name="outst", bufs=8))
        out_stage = [
            out_pool.tile([P, HOUT], F32, tag="outst", name=f"outst{i}")
            for i in range(8)
        ]
        pt_pool = ctx.enter_context(tc.tile_pool(name="pT", bufs=18))
        ctt_pool = ctx.enter_context(tc.tile_pool(name="ctt", bufs=4))
        fin_pool = ctx.enter_context(tc.tile_pool(name="fin", bufs=4))
        spsum = ctx.enter_context(tc.tile_pool(name="spsum", bufs=6, space="PSUM"))
        vpsum = ctx.enter_context(tc.tile_pool(name="vpsum", bufs=2, space="PSUM"))

        out_perm = out_ap.rearrange("(p r) c -> p r c", p=P)

        def emit_finalize(h, ctxT_sb):
            """Epilogue for head h: back-transpose the fp32 ctx^T staging
            tiles to [q, 65] on the PE, divide by the denominator. Deferred
            one head so the in-order PE stream never stalls waiting on it."""
            ctx_ps = [
                spsum.tile([P, 512], F32, tag="sT", name=f"ctx{h}_{i}")
                for i in range(2)
            ]
            # all PE transposes first, then all DVE reads: avoids the
            # per-slot PE-write/DVE-read same-bank ping-pong serialization
            for qt in range(QT):
                cp = ctx_ps[qt // 4]
                sl = (qt % 4) * VW
                nc.tensor.transpose(
                    cp[:, sl:sl + VW],
                    ctxT_sb[qt // 4][:, (qt % 4) * P:(qt % 4 + 1) * P],
                    ident32[:VW, :VW],
                )
            for qt in range(QT):
                cp = ctx_ps[qt // 4]
                sl = (qt % 4) * VW
                rc = fin_pool.tile([P, 1], F32, tag="recip")
                nc.vector.reciprocal(rc[:], cp[:, sl + HD:sl + HD + 1])
                nc.scalar.activation(
                    out_stage[qt][:, h * HD:(h + 1) * HD],
                    cp[:, sl:sl + HD],
                    AF.Identity,
                    scale=rc[:],
                )
                if h == NH - 1:
                    nc.sync.dma_start(out_perm[:, qt, :], out_stage[qt][:])

        pending_fin = None
        for h in range(NH):
            halves = rel_strips.pop(h)
            if h + 2 < NH:
                rel_dmas[h + 2] = emit_rel_dmas(h + 2)

            def strip(qt):
                return halves[qt // 4][:, qt % 4, :]

            dt, rem = divmod(h, 2)
            d0 = rem * HD
            qTh = qT[dt][d0:d0 + HD, :]
            kTh = kT[dt][d0:d0 + HD, :]

            pT_strips = [
                pt_pool.tile([P, S], F16, tag="pT", name=f"pT{h}_{kt}")
                for kt in range(KT)
            ]
            # scores^T, qch-outer: the first half of the head needs only
            # rel half-tile 0 (half 1 streams in during the first half)
            for qch in range(2):
                for kt in range(KT):
                    ps = spsum.tile([P, 512], F32, tag="sT")
                    nc.tensor.matmul(
                        ps[:],
                        kTh[:, kt * P:(kt + 1) * P],
                        qTh[:, qch * 512:(qch + 1) * 512],
                        start=True,
                        stop=False,
                    )
                    # += rel12^T (transposing adds via fp16 identity rhs)
                    for j in range(4):
                        qt = qch * 4 + j
                        nc.tensor.matmul(
                            ps[:, j * P:(j + 1) * P],
                            strip(qt)[:, kt * P:(kt + 1) * P],
                            ident[:],
                            start=False,
                            stop=(j == 3),
                        )
                    # exp(scores + mask bias) -> fp16 probs
                    nc.scalar.activation(
                        pT_strips[kt][:, qch * 512:(qch + 1) * 512],
                        ps[:],
                        AF.Exp,
                        bias=maskb[:, kt:kt + 1],
                        scale=1.0,
                    )
                if qch == 0:
                    if pending_fin is not None:
                        emit_finalize(*pending_fin)
                        pending_fin = None
                    # merge adds for head h+1, emitted AFTER the finalize
                    # recips so they can't block them in the DVE FIFO
                    if h + 1 < NH and h + 1 not in rel_strips:
                        rel_strips[h + 1] = emit_rel_adds(
                            rel_dmas.pop(h + 1)
                        )

            # PV flipped: ctx^T[d|1, q] = V_aug.T @ P^T, accumulated over kt.
            # Row 64 of ctx^T is the softmax denominator (ones col of V_aug).
            ctxT_ps = [
                vpsum.tile([VW, 512], F32, tag="ctxT", name=f"ctxT{h}_{i}")
                for i in range(2)
            ]
            for qch in range(2):
                for kt in range(KT):
                    nc.tensor.matmul(
                        ctxT_ps[qch][:],
                        v_tiles[kt][:, h, :],
                        pT_strips[kt][:, qch * 512:(qch + 1) * 512],
                        start=(kt == 0),
                        stop=(kt == KT - 1),
                    )
            # drain the 2 ctx^T PSUM banks to SBUF eagerly so they recycle
            ctxT_sb = []
            for qch in range(2):
                t_ = ctt_pool.tile([VW, 512], F32, tag="ctxT_sb",
                                   name=f"ctxTs{h}_{qch}")
                nc.scalar.copy(t_[:], ctxT_ps[qch][:])
                ctxT_sb.append(t_)
            pending_fin = (h, ctxT_sb)

        emit_finalize(*pending_fin)


def build_program():
    """Build and compile the per-core Bass program. Returns nc."""
    nc = bacc.Bacc(
        "TRN2",
        target_bir_lowering=False,
        debug=False,
        num_devices=8,
    )
    aps = {
        "x": nc.dram_tensor("x", [S, HIN], F32, kind="ExternalInput").ap(),
        "mask": nc.dram_tensor("mask", [S], I32, kind="ExternalInput").ap(),
        "rel1": nc.dram_tensor("rel1", [NH, S, S], F32, kind="ExternalInput").ap(),
        "rel2": nc.dram_tensor("rel2", [NH, S, S], F32, kind="ExternalInput").ap(),
        "wq": nc.dram_tensor("wq", [HOUT, HIN], F32, kind="ExternalInput").ap(),
        "wk": nc.dram_tensor("wk", [HOUT, HIN], F32, kind="ExternalInput").ap(),
        "wv": nc.dram_tensor("wv", [HOUT, HIN], F32, kind="ExternalInput").ap(),
        "bq": nc.dram_tensor("bq", [HOUT], F32, kind="ExternalInput").ap(),
        "bk": nc.dram_tensor("bk", [HOUT], F32, kind="ExternalInput").ap(),
        "bv": nc.dram_tensor("bv", [HOUT], F32, kind="ExternalInput").ap(),
        "out": nc.dram_tensor("out", [S, HOUT], F32, kind="ExternalOutput").ap(),
    }
    with tile.TileContext(nc) as tc:
        _build_kernel_body(tc, aps)
    nc.compile()
    return nc


_KPERM = (np.arange(R)[:, None] + R * np.arange(P)[None, :]).ravel()


def make_in_maps(inputs):
    """Slice full inputs into the 8 per-core input maps.

    rel tensors get their k-axis pre-permuted to the on-chip layout
    (position kt*128+j holds key 8j+kt), so the kernel's contiguous
    k-slices line up with the permuted K^T/V tiles."""
    hs = np.ascontiguousarray(np.asarray(inputs["hidden_states"], np.float32))
    am = np.asarray(inputs["attention_mask"]).astype(np.int32)
    rel1 = np.asarray(inputs["rel_pos"], np.float32)[:, :, :, _KPERM]
    rel2 = np.asarray(inputs["rel_2d_pos"], np.float32)[:, :, :, _KPERM]
    ws = {k: np.asarray(inputs["W" + k[-1]], np.float32) for k in ("wq", "wk", "wv")}
    bs = {k: np.asarray(inputs["b" + k[-1]], np.float32) for k in ("bq", "bk", "bv")}

    in_maps = []
    for c in range(8):
        b, hh = divmod(c, 2)
        hsl = slice(hh * NH, (hh + 1) * NH)
        csl = slice(hh * HOUT, (hh + 1) * HOUT)
        m = {
            "x": np.ascontiguousarray(hs[b]),
            "mask": np.ascontiguousarray(am[b, 0, 0]),
            "rel1": np.ascontiguousarray(rel1[b, hsl]),
            "rel2": np.ascontiguousarray(rel2[b, hsl]),
        }
        for k in ("wq", "wk", "wv"):
            m[k] = np.ascontiguousarray(ws[k][csl])
        for k in ("bq", "bk", "bv"):
            m[k] = np.ascontiguousarray(bs[k][csl])
        in_maps.append(m)
    return in_maps


def gather_output(results):
    out = np.empty((4, S, HIN), np.float32)
    for c in range(8):
        b, hh = divmod(c, 2)
        out[b, :, hh * HOUT:(hh + 1) * HOUT] = results[c]["out"]
    return out


_NC_CACHE = []


def kernel(**inputs):
    if not _NC_CACHE:
        _NC_CACHE.append(build_program())
    nc = _NC_CACHE[0]
    in_maps = make_in_maps(inputs)
    res = run_bass_kernel_spmd(nc, in_maps, list(range(8)))
    return gather_output(res.results)
